# revision 1
# baseline (speedup 1.0000x reference)
"""GAT layer (nn_GATLayer_24249385353673) Trainium2 Bass kernel.

Sharding: data-parallel over batch b — core b computes batch element b.
No collectives. Each core:
  Wh = h_b @ W            [1024, 256]  (+ e1/e2 via extended weight matrix)
  P_T[j,i] = exp(lrelu(e1_i + e2_j) + maskbias[j,i])   (maskbias in {0,-1000})
  outT[d,i] = sum_j Wh[j,d] * P_T[j,i]   (+ ones column -> denom row)
  out[i,d]  = outT[d,i] / denom[i]       (host transposes outT at gather)

Shapes hardcoded: B=8, N=1024, D_IN=256, D_OUT=256, H=8, HD=32, ALPHA=0.2.
"""

import os
from contextlib import ExitStack

import numpy as np

B, N, D_IN, D_OUT, H, HD = 8, 1024, 256, 256, 8, 32
ALPHA = 0.2
NEG_MASK = -1000.0  # additive pre/post-lrelu mask value (exp -> exactly 0)
SHIFT = -4.0  # post-lrelu shift so exp() stays in fp16 range
N_CORES = 8
NC_CHUNKS = N // 128  # 8 node chunks of 128

# Per-head pipeline choice: heads 0..ACT_PATH_HEADS-1 use the scalar-engine
# Prelu+Exp path (e2 rides the per-partition bias); the rest use the vector-
# engine lrelu chain. GPS_MAX_HEADS of the DVE-path heads run their max() on
# the otherwise-idle GPSIMD engine. Tuned against the cost model.
# Interleave scalar-engine-path and vector-engine-path heads so both
# engines stay busy through the whole main loop.
ACT_PATH_SET = (0, 2, 4, 6)

_NC_CACHE = {}
LAST_RESULT = None  # BassKernelResults of the most recent run (for test.py)


def _register_custom_op():
    """Fused DVE op: out = lrelu(in1 + s0) + in0 + s1
    (in1 = e1 broadcast tile, s0 = e2 per-partition column, in0 = mask bias,
     s1 = constant shift, imm2 = leaky slope)."""
    import concourse.dve_ops as dve_ops_mod
    from concourse.dve_ops import DveOp
    from concourse.dve_spec import C0, C1, C2, Spec, Src0, Src1, lower, maxx
    from concourse.dve_table_gen import dve_ver_for
    from concourse.dve_uop import DveOpSpec

    name = "GAT_SCORE_ANT"
    if name in dve_ops_mod._SUB_OPCODE_FOR_NAME:
        return next(op for op in dve_ops_mod.OPS if op.name == name)

    _t = Src1 + C0
    spec = Spec(
        body=maxx(_t, _t * C2) + Src0 + C1,
        reference=lambda in0, in1, s0, s1, imm2: (
            np.maximum(in1.astype(np.float32) + s0, (in1.astype(np.float32) + s0) * imm2)
            + in0
            + s1
        ).astype(np.float32),
    )
    op = DveOp(name, spec, subdim=False, uops_sha={})
    row = max(dve_ops_mod._SUB_OPCODE_FOR_NAME.values()) + 1
    assert row < 0x20
    dve_ops_mod.OPS.append(op)
    dve_ops_mod._SUB_OPCODE_FOR_NAME[name] = row
    dve_ops_mod.CUSTOM_DVE_SPECS[name] = spec
    for trn in ("TRN2", "TRN3"):
        ver = dve_ver_for(trn)
        if ver in op.uops_sha:
            continue
        compiled = DveOpSpec(
            name=name, opcode=row, uops=lower(spec, ver=ver), rd1_en=True
        )
        op.uops_sha[ver] = compiled.sha(ver)
    return op


def _patch_tile_drain():
    """This container's walrus build only encodes ONE sync wait per
    instruction; Tile's kernel-tail drain carries one wait per live
    semaphore. Split the waits across follow-up sync-engine nops."""
    import concourse.tile as tile
    from concourse.vector_clock import ScopedClock

    if getattr(tile.TileContext, "_gat_drain_patched", False):
        return

    def _drain_and_barrier(self, tick_clock, wait_clock):
        nc = self.nc
        drain_inst = nc.sync.drain()
        wait_clock.add_sem_waits(
            drain_inst.ins, ScopedClock({None: tick_clock.global_clock})
        )
        si = drain_inst.ins.sync_info
        waits = list(si.on_wait)
        if len(waits) > 1:
            si.on_wait = waits[:1]
            drain_inst.ins.sync_info = si
            si_cls = type(si)
            for w in waits[1:]:
                nop = nc.sync.nop()
                nop.ins.sync_info = si_cls(on_wait=[w], on_update=[])
        nc.all_engine_barrier()
        assert self.sems is not None
        popped = nc._tile_sem_poison_stack.pop()
        assert popped is self._sem_poison
        nc.clear_and_free_semaphores(list(self.sems.allocated().values()))
        nc.all_engine_barrier()

    tile.TileContext._drain_and_barrier = _drain_and_barrier
    tile.TileContext._gat_drain_patched = True


def _split_multi_waits(nc):
    """This walrus build encodes at most ONE sync wait per instruction.
    Move excess waits onto same-engine NoOps inserted just before the
    offending instruction (engines execute their stream in order, so
    hoisting waits to earlier slots on the same engine is equivalent)."""
    import concourse.mybir as mybir

    si_cls = None
    n_new = 0
    for f in nc.m.functions:
        for bb in f.blocks:
            insts = bb.instructions
            out = []
            for inst in insts:
                si = inst.sync_info
                waits = list(si.on_wait) if si is not None else []
                if len(waits) > 1:
                    if si_cls is None:
                        si_cls = type(si)
                    for w in waits[:-1]:
                        nop = mybir.InstNoOp(
                            name=f"waitnop-{n_new}",
                            ins=[],
                            outs=[],
                            engine=inst.engine,
                        )
                        nop.sync_info = si_cls(on_wait=[w], on_update=[])
                        out.append(nop)
                        n_new += 1
                    si.on_wait = waits[-1:]
                    inst.sync_info = si
                out.append(inst)
            if n_new:
                insts[:] = out
    return n_new


def _build_nc(split_waits=True):
    import concourse.bass as bass
    import concourse.mybir as mybir
    import concourse.tile as tile
    from concourse.masks import make_identity

    _patch_tile_drain()

    f32 = mybir.dt.float32
    f16 = mybir.dt.float16
    AF = mybir.ActivationFunctionType

    nc = bass.Bass()
    htb_d = nc.dram_tensor("htb", [D_IN, N], mybir.dt.bfloat16, kind="ExternalInput")
    wexb_d = nc.dram_tensor(
        "wexb", [D_IN, D_OUT + 2 * H], mybir.dt.bfloat16, kind="ExternalInput"
    )
    mb_d = nc.dram_tensor("mb", [N, N], f16, kind="ExternalInput")
    outT_d = nc.dram_tensor("outT", [D_OUT, N], f32, kind="ExternalOutput")
    e1_scratch = nc.dram_tensor("e1_scratch", [1, H * N], f16)

    with tile.TileContext(nc) as tc, ExitStack() as ctx:
        const = ctx.enter_context(tc.tile_pool(name="const", bufs=1))
        ident = const.tile([128, 128], f32)
        make_identity(nc, ident[:])
        ones_row = const.tile([1, 128], f16, tag="ones_row")
        nc.vector.memset(ones_row[:], 1.0)
        # blockones[k, 32k:32(k+1)] = 1 — for replicating 4 rows to 128 parts
        shift_col = const.tile([128, 1], f32, tag="shift_col")
        nc.vector.memset(shift_col[:], SHIFT)
        blockones = const.tile([4, 128], f32, tag="blockones")
        ones32f = const.tile([1, 32], f32, tag="ones32f")
        nc.vector.memset(blockones[:], 0.0)
        nc.vector.memset(ones32f[:], 1.0)
        for k in range(4):
            nc.gpsimd.dma_start(
                blockones[k : k + 1, 32 * k : 32 * (k + 1)], ones32f[:]
            )

        h_pool = ctx.enter_context(tc.tile_pool(name="h", bufs=1))
        wex_pool = ctx.enter_context(tc.tile_pool(name="wex", bufs=1))
        mb_pool = ctx.enter_context(tc.tile_pool(name="mb", bufs=1))
        ht_pool = ctx.enter_context(tc.tile_pool(name="ht", bufs=2))
        wh_pool = ctx.enter_context(tc.tile_pool(name="wh", bufs=NC_CHUNKS))
        e_pool = ctx.enter_context(tc.tile_pool(name="e", bufs=NC_CHUNKS))
        e1r_pool = ctx.enter_context(tc.tile_pool(name="e1r", bufs=1))
        e1b_pool = ctx.enter_context(tc.tile_pool(name="e1b", bufs=1))
        y_pool = ctx.enter_context(tc.tile_pool(name="y", bufs=8))
        w_pool = ctx.enter_context(tc.tile_pool(name="w", bufs=8))
        t_pool = ctx.enter_context(tc.tile_pool(name="t", bufs=8))
        p_pool = ctx.enter_context(tc.tile_pool(name="p", bufs=8))
        outt_pool = ctx.enter_context(tc.tile_pool(name="outt", bufs=2))
        dn_pool = ctx.enter_context(tc.tile_pool(name="dn", bufs=1))
        rn_pool = ctx.enter_context(tc.tile_pool(name="rn", bufs=1))
        of_pool = ctx.enter_context(tc.tile_pool(name="of", bufs=2))

        # ---- DMA inputs in (single batched DMA per tensor, bf16) ----
        bf16 = mybir.dt.bfloat16
        htb_all = ht_pool.tile([128, 2, N], bf16, tag="htb")
        nc.sync.dma_start(
            htb_all[:], htb_d[:].rearrange("(k p) d -> p k d", p=128)
        )
        htb_sb = [htb_all[:, kc, :] for kc in range(2)]
        wexb_all = wex_pool.tile([128, 2, D_OUT + 2 * H], bf16, tag="wexb")
        nc.sync.dma_start(
            wexb_all[:], wexb_d[:].rearrange("(k p) d -> p k d", p=128)
        )
        wexb_sb = [wexb_all[:, kc, :] for kc in range(2)]
        mb_all = mb_pool.tile([128, NC_CHUNKS, N], f16, tag="mb")
        nc.sync.dma_start(
            mb_all[:], mb_d[:].rearrange("(c p) d -> p c d", p=128)
        )
        mb_sb = [mb_all[:, c, :] for c in range(NC_CHUNKS)]

        # warm the exp activation table early (overlaps with DMAs)
        warm = const.tile([1, 8], f32, tag="warm")
        nc.vector.memset(warm[:], 0.0)
        nc.scalar.activation(warm[:], warm[:], AF.Exp)

        # ---- matmul1: e1/e2 columns FIRST (tiny, unblocks the e1 row
        # broadcast chain), then the full Wh ----
        wh_sb = []  # [128, H, HD+1] fp16 per node chunk: [Wh_head | ones]
        e_sb = []  # [128, 16] f32 per node chunk: cols 0:8 e1, 8:16 e2
        with tc.tile_pool(name="psum_e", bufs=2, space="PSUM") as psE, tc.tile_pool(
            name="psum_mm1", bufs=2, space="PSUM"
        ) as psB, tc.tile_pool(name="psum_e1t", bufs=1, space="PSUM") as psT:
            for c in range(NC_CHUNKS):
                pe_ = psE.tile([128, 2 * H], f32, tag="mme", name=f"mme{c}")
                for kc in range(2):
                    nc.tensor.matmul(
                        pe_[:],
                        htb_sb[kc][:, c * 128 : (c + 1) * 128],
                        wexb_sb[kc][:, D_OUT : D_OUT + 2 * H],
                        start=(kc == 0),
                        stop=(kc == 1),
                    )
                et = e_pool.tile([128, 2 * H], f32, tag="e")
                nc.vector.tensor_copy(et[:], pe_[:])
                e_sb.append(et)
            # e1 rows: transpose e1 columns -> [8, 1024], then fp16
            e1t = psT.tile([8, N], f32, tag="e1t")
            for c in range(NC_CHUNKS):
                nc.tensor.transpose(
                    e1t[:, c * 128 : (c + 1) * 128], e_sb[c][:, 0:H], ident[:]
                )
            e1r = e1r_pool.tile([8, N], f16, tag="e1r")
            nc.vector.tensor_copy(e1r[:], e1t[:])
            nc.sync.dma_start(e1_scratch[:], e1r[:])
            for c in range(NC_CHUNKS):
                p1 = psB.tile([128, D_OUT], f32, tag="mm1")
                for kc in range(2):
                    nc.tensor.matmul(
                        p1[:],
                        htb_sb[kc][:, c * 128 : (c + 1) * 128],
                        wexb_sb[kc][:, 0:D_OUT],
                        start=(kc == 0),
                        stop=(kc == 1),
                    )
                wt = wh_pool.tile([128, H, HD + 1], f16, tag="wh")
                nc.vector.tensor_copy(
                    wt[:, :, 0:HD], p1[:].rearrange("p (h q) -> p h q", h=H)
                )
                nc.vector.memset(wt[:, :, HD : HD + 1], 1.0)
                wh_sb.append(wt)



        # ---- main loop: scores -> exp -> attention matmul ----
        outt_sb = [outt_pool.tile([128, N], f32, tag="outt", name=f"outt{i}") for i in range(2)]
        dn128 = dn_pool.tile([128, N // 16], f32, tag="dn")
        stage_pool = ctx.enter_context(tc.tile_pool(name="stage", bufs=2))
        # Keep the PE busy across the pre-main-loop lull: a >3.4us idle
        # window re-throttles the PE clock to 1.2GHz (HAM), and the main
        # loop's ~65% PE duty can never re-warm it. These fillers bridge
        # the gap so attention matmuls run at full clock.
        with tc.tile_pool(name="psum_warm", bufs=1, space="PSUM") as psW:
            warm_ps = psW.tile([128, 512], f32, tag="warm_ps")
            for _ in range(16):
                nc.tensor.matmul(
                    warm_ps[:],
                    htb_sb[0][:, 0:128],
                    htb_sb[0][:, 0:512],
                    start=True,
                    stop=True,
                )

        # e1 broadcast: zero-stride DRAM reads replicate each head's row
        # across all 128 partitions (one DMA per head so head 0 unblocks fast)
        e1b_all = e1b_pool.tile([128, H * N], f16, tag="e1b", name="e1b_all")
        for hh in range(H):
            nc.sync.dma_start(
                e1b_all[:, hh * N : (hh + 1) * N],
                e1_scratch[0:1, hh * N : (hh + 1) * N].partition_broadcast(128),
            )
        e1b_sb = [e1b_all[:, hh * N : (hh + 1) * N] for hh in range(H)]
        with tc.tile_pool(name="psum_mm2", bufs=6, space="PSUM") as ps2:
            warm2 = ps2.tile([128, 256], f32, tag="warm2", bufs=1)
            for hh in range(H):
                e1b = e1b_sb[hh]
                acc = [ps2.tile([HD + 1, 512], f32, tag="mm2", name=f"acc{hh}_{i}") for i in range(2)]
                for jc in range(NC_CHUNKS):
                    e2col = e_sb[jc][:, H + hh : H + hh + 1]
                    p = p_pool.tile([128, N], f16, tag="p")
                    if hh in ACT_PATH_SET:
                        # u = e1 + mb (DVE 2x); v = prelu(u + e2) (ACT);
                        # p = exp(v + shift) (ACT)
                        u = y_pool.tile([128, N], f16, tag="y")
                        nc.vector.tensor_tensor(
                            out=u[:], in0=e1b[:], in1=mb_sb[jc][:],
                            op=mybir.AluOpType.add,
                        )
                        v = w_pool.tile([128, N], f16, tag="w")
                        nc.scalar.activation(
                            v[:], u[:], AF.Prelu, bias=e2col, alpha=ALPHA
                        )
                        nc.scalar.activation(p[:], v[:], AF.Exp, bias=shift_col[:])
                    else:
                        # u = (mb + e2) + e1 (DVE 1x); t = alpha*u (DVE 4x);
                        # w = max(u, t) (DVE 2x); p = exp(w + shift)
                        u = y_pool.tile([128, N], f16, tag="y")
                        nc.vector.scalar_tensor_tensor(
                            out=u[:], in0=mb_sb[jc][:], scalar=e2col,
                            in1=e1b[:], op0=mybir.AluOpType.add,
                            op1=mybir.AluOpType.add,
                        )
                        t = t_pool.tile([128, N], f16, tag="t")
                        nc.vector.tensor_scalar_mul(t[:], u[:], ALPHA)
                        w = w_pool.tile([128, N], f16, tag="w")
                        nc.vector.tensor_tensor(
                            out=w[:], in0=u[:], in1=t[:], op=mybir.AluOpType.max
                        )
                        nc.scalar.activation(p[:], w[:], AF.Exp, bias=shift_col[:])
                    for ic in range(2):
                        nc.tensor.matmul(
                            acc[ic][:],
                            wh_sb[jc][:, hh, :],
                            p[:, ic * 512 : (ic + 1) * 512],
                            start=(jc == 0),
                            stop=(jc == NC_CHUNKS - 1),
                        )
                    # tiny filler keeps the PE's activity monitor from
                    # re-throttling the clock during sub-window idle gaps
                    nc.tensor.matmul(
                        warm2[:],
                        htb_sb[0][:, 0:128],
                        htb_sb[0][:, 0:256],
                        start=True,
                        stop=True,
                    )
                # evacuate PSUM -> SBUF staging (scalar engine), then DMA
                # remap partitions: numerator rows -> outT chunk, denom -> dn
                g, k = hh // 4, hh % 4
                stage = stage_pool.tile([HD + 1, N], f32, tag="stage", name=f"st{hh}")
                nc.scalar.copy(stage[:, 0:512], acc[0][:])
                nc.vector.tensor_copy(stage[:, 512:1024], acc[1][:])
                nc.sync.dma_start(
                    outt_sb[g][k * HD : (k + 1) * HD, :], stage[0:HD, :]
                )
                nc.sync.dma_start(
                    dn128[hh * 16 : (hh + 1) * 16, :], stage[HD : HD + 1, :]
                )
                if k == 3:
                    # group g (heads 4g..4g+3) done: normalize + write out now,
                    # overlapping the next group's main loop
                    rn128 = rn_pool.tile(
                        [64, N // 16], f32, tag=f"rn128_{g}", name=f"rn128_{g}"
                    )
                    nc.vector.reciprocal(
                        rn128[:], dn128[g * 64 : (g + 1) * 64, :]
                    )
                    rn0 = rn_pool.tile([4, N], f32, tag=f"rn0_{g}", name=f"rn0_{g}")
                    nc.sync.dma_start(rn0[:], rn128[:])
                    rb = ps2.tile([128, 512], f32, tag="rb", name=f"rb{g}", bufs=1)
                    of = of_pool.tile([128, N], f32, tag="of", name=f"of{g}")
                    for ic in range(2):
                        nc.tensor.matmul(
                            rb[:],
                            blockones[:],
                            rn0[:, ic * 512 : (ic + 1) * 512],
                            start=True,
                            stop=True,
                        )
                        nc.vector.tensor_mul(
                            of[:, ic * 512 : (ic + 1) * 512],
                            outt_sb[g][:, ic * 512 : (ic + 1) * 512],
                            rb[:],
                        )
                        nc.sync.dma_start(
                            outT_d[g * 128 : (g + 1) * 128, ic * 512 : (ic + 1) * 512],
                            of[:, ic * 512 : (ic + 1) * 512],
                        )

    if split_waits:
        _split_multi_waits(nc)
    return nc


def _get_nc():
    if "nc" not in _NC_CACHE:
        _NC_CACHE["nc"] = _build_nc()
    return _NC_CACHE["nc"]


def _prep_inputs(h, adj_mask, W, a):
    hT = np.ascontiguousarray(np.swapaxes(np.asarray(h, dtype=np.float32), 1, 2))
    adj = np.asarray(adj_mask)
    W = np.asarray(W, dtype=np.float32)
    a = np.asarray(a, dtype=np.float32)

    # maskbias, transposed: mb[b, j, i] = 0 if adj[b, i, j] else NEG_MASK
    mb = np.where(
        np.swapaxes(adj, 1, 2) == 0, np.float16(NEG_MASK), np.float16(0.0)
    ).astype(np.float16)  # added BEFORE lrelu; SHIFT applied in the exp bias

    Wr = W.reshape(D_IN, H, HD)
    w1 = Wr @ a[:HD]  # [D_IN, H]
    w2 = Wr @ a[HD:]  # [D_IN, H]
    wex = np.ascontiguousarray(
        np.concatenate([W, w1, w2], axis=1), dtype=np.float32
    )
    import ml_dtypes
    htb = hT.astype(ml_dtypes.bfloat16)
    wexb = wex.astype(ml_dtypes.bfloat16)
    return mb, htb, wexb


def kernel(h, adj_mask, W, a):
    global LAST_RESULT
    # persistent jax/XLA cache: repeat calls (and reruns) skip the multi-
    # minute neuronx-cc compile for an unchanged module
    os.environ.setdefault("JAX_COMPILATION_CACHE_DIR", "/tmp/jax_bass_cache")
    from concourse.bass_utils import run_bass_kernel_spmd

    mb_np, htb_np, wexb_np = _prep_inputs(h, adj_mask, W, a)
    nc = _get_nc()

    core_ids = list(range(N_CORES))
    in_maps = [
        {
            "htb": np.ascontiguousarray(htb_np[b]),
            "mb": np.ascontiguousarray(mb_np[b]),
            "wexb": wexb_np,
        }
        for b in range(N_CORES)
    ]
    res = run_bass_kernel_spmd(nc, in_maps, core_ids)
    LAST_RESULT = res
    out = np.stack(
        [np.ascontiguousarray(res.results[b]["outT"].T) for b in range(N_CORES)]
    ).astype(np.float32)
    return out



# revision 5
# speedup vs baseline: 1.4230x; 1.4230x over previous
"""GAT layer (nn_GATLayer_24249385353673) Trainium2 Bass kernel.

Sharding: data-parallel over batch b — core b computes batch element b.
No collectives.

Algebra: exp(lrelu(e1_i + e2_j)) = exp(e1_i) * max(r_j*t_i, u_j) with
  t_i = exp(-0.8*e1_i), r_j = exp(0.2*e2_j + SHIFT), u_j = exp(e2_j + SHIFT).
The exp(e1_i) column factor cancels in the softmax ratio, so each core only
needs, per (head, j-chunk):
  Q = max(t_bcast * r_j, u_j)        one tensor_scalar  (DVE 4x mode)
  G = min(Q, af)   af in {0, 1000}   one tensor_tensor  (DVE 2x / GPSIMD)
  acc[33, 512] += whT[j, 33] @ G     (col 0 of wh is ones -> denominator row)
Numerator/denominator ship to the host unnormalized (fp16); the host divides.

Shapes hardcoded: B=8, N=1024, D_IN=256, D_OUT=256, H=8, HD=32, ALPHA=0.2.
"""

import os
from contextlib import ExitStack

import numpy as np

B, N, D_IN, D_OUT, H, HD = 8, 1024, 256, 256, 8, 32
ALPHA = 0.2
SHIFT = -4.0  # folded into u/r exps; scales num+den equally, keeps fp16 safe
N_CORES = 8
NC_CHUNKS = N // 128  # 8 node chunks of 128

# GPSIMD cannot run TensorTensor in this walrus build (ISA engine check
# rejects Pool), so every mask min() runs on DVE.
def _mask_on_gpsimd(hh, c):
    return False

_NC_CACHE = {}
LAST_RESULT = None  # BassKernelResults of the most recent run (for test.py)


def _patch_tile_drain():
    """This container's walrus build only encodes ONE sync wait per
    instruction; Tile's kernel-tail drain carries one wait per live
    semaphore. Split the waits across follow-up sync-engine nops."""
    import concourse.tile as tile
    from concourse.vector_clock import ScopedClock

    if getattr(tile.TileContext, "_gat_drain_patched", False):
        return

    def _drain_and_barrier(self, tick_clock, wait_clock):
        nc = self.nc
        drain_inst = nc.sync.drain()
        wait_clock.add_sem_waits(
            drain_inst.ins, ScopedClock({None: tick_clock.global_clock})
        )
        si = drain_inst.ins.sync_info
        waits = list(si.on_wait)
        if len(waits) > 1:
            si.on_wait = waits[:1]
            drain_inst.ins.sync_info = si
            si_cls = type(si)
            for w in waits[1:]:
                nop = nc.sync.nop()
                nop.ins.sync_info = si_cls(on_wait=[w], on_update=[])
        nc.all_engine_barrier()
        assert self.sems is not None
        popped = nc._tile_sem_poison_stack.pop()
        assert popped is self._sem_poison
        nc.clear_and_free_semaphores(list(self.sems.allocated().values()))
        nc.all_engine_barrier()

    tile.TileContext._drain_and_barrier = _drain_and_barrier
    tile.TileContext._gat_drain_patched = True


def _split_multi_waits(nc):
    """This walrus build encodes at most ONE sync wait per instruction.
    Move excess waits onto same-engine NoOps inserted just before the
    offending instruction (engines execute their stream in order, so
    hoisting waits to earlier slots on the same engine is equivalent)."""
    import concourse.mybir as mybir

    si_cls = None
    n_new = 0
    for f in nc.m.functions:
        for bb in f.blocks:
            insts = bb.instructions
            out = []
            for inst in insts:
                si = inst.sync_info
                waits = list(si.on_wait) if si is not None else []
                if len(waits) > 1:
                    if si_cls is None:
                        si_cls = type(si)
                    for w in waits[:-1]:
                        nop = mybir.InstNoOp(
                            name=f"waitnop-{n_new}",
                            ins=[],
                            outs=[],
                            engine=inst.engine,
                        )
                        nop.sync_info = si_cls(on_wait=[w], on_update=[])
                        out.append(nop)
                        n_new += 1
                    si.on_wait = waits[-1:]
                    inst.sync_info = si
                out.append(inst)
            if n_new:
                insts[:] = out
    return n_new


def _build_nc(split_waits=True):
    import concourse.bass as bass
    import concourse.mybir as mybir
    import concourse.tile as tile
    from concourse.masks import make_identity

    _patch_tile_drain()

    f32 = mybir.dt.float32
    f16 = mybir.dt.float16
    bf16 = mybir.dt.bfloat16
    AF = mybir.ActivationFunctionType
    Alu = mybir.AluOpType

    nc = bass.Bass()
    htb_d = nc.dram_tensor("htb", [D_IN, N], bf16, kind="ExternalInput")
    # [W | w1 | w2 | alpha*w2]
    wexb_d = nc.dram_tensor(
        "wexb", [D_IN, D_OUT + 3 * H], bf16, kind="ExternalInput"
    )
    af_d = nc.dram_tensor("af", [N, N], f16, kind="ExternalInput")
    outd_d = nc.dram_tensor("outd", [H * (HD + 1), N], f16, kind="ExternalOutput")
    t_scratch = nc.dram_tensor("t_scratch", [1, H * N], f16)

    with tile.TileContext(nc) as tc, ExitStack() as ctx:
        const = ctx.enter_context(tc.tile_pool(name="const", bufs=1))
        ident = const.tile([128, 128], f32)
        make_identity(nc, ident[:])
        shift_col = const.tile([128, 1], f32, tag="shift_col")
        nc.vector.memset(shift_col[:], SHIFT)

        h_pool = ctx.enter_context(tc.tile_pool(name="h", bufs=1))
        wex_pool = ctx.enter_context(tc.tile_pool(name="wex", bufs=1))
        af_pool = ctx.enter_context(tc.tile_pool(name="af", bufs=1))
        tb_pool = ctx.enter_context(tc.tile_pool(name="tb", bufs=1))
        wh_pool = ctx.enter_context(tc.tile_pool(name="wh", bufs=NC_CHUNKS))
        et_pool = ctx.enter_context(tc.tile_pool(name="et", bufs=NC_CHUNKS))
        eu_pool = ctx.enter_context(tc.tile_pool(name="eu", bufs=NC_CHUNKS))
        tr_pool = ctx.enter_context(tc.tile_pool(name="tr", bufs=1))
        q_pool = ctx.enter_context(tc.tile_pool(name="q", bufs=4))
        g_pool = ctx.enter_context(tc.tile_pool(name="g", bufs=4))
        st_pool = ctx.enter_context(tc.tile_pool(name="st", bufs=2))

        # ---- DMA inputs in (single batched DMA per tensor) ----
        htb_all = h_pool.tile([128, 2, N], bf16, tag="htb")
        nc.sync.dma_start(htb_all[:], htb_d[:].rearrange("(k p) d -> p k d", p=128))
        htb_sb = [htb_all[:, kc, :] for kc in range(2)]
        wexb_all = wex_pool.tile([128, 2, D_OUT + 3 * H], bf16, tag="wexb")
        nc.sync.dma_start(
            wexb_all[:], wexb_d[:].rearrange("(k p) d -> p k d", p=128)
        )
        wexb_sb = [wexb_all[:, kc, :] for kc in range(2)]
        af_all = af_pool.tile([128, NC_CHUNKS, N], f16, tag="af")
        nc.sync.dma_start(af_all[:], af_d[:].rearrange("(c p) d -> p c d", p=128))
        af_sb = [af_all[:, c, :] for c in range(NC_CHUNKS)]

        # warm the exp activation table early (overlaps with DMAs)
        warm = const.tile([1, 8], f32, tag="warm")
        nc.vector.memset(warm[:], 0.0)
        nc.scalar.activation(warm[:], warm[:], AF.Exp)

        # ---- matmul1: e columns first (tiny; unblocks the t-row broadcast
        # chain), then the full Wh ----
        et_sb = []  # [128, 3H] f32 per chunk: e1 | e2 | 0.2*e2
        eu_sb = []  # [128, 2H] f32 per chunk: u = exp(e2+S) | r = exp(0.2*e2+S)
        wh_sb = []  # [128, H, HD+1] f16 per chunk: [ones | Wh_head]
        with tc.tile_pool(name="psum_e", bufs=2, space="PSUM") as psE, tc.tile_pool(
            name="psum_mm1", bufs=2, space="PSUM"
        ) as psB, tc.tile_pool(name="psum_e1t", bufs=1, space="PSUM") as psT:
            for c in range(NC_CHUNKS):
                pe_ = psE.tile([128, 3 * H], f32, tag="mme", name=f"mme{c}")
                for kc in range(2):
                    nc.tensor.matmul(
                        pe_[:],
                        htb_sb[kc][:, c * 128 : (c + 1) * 128],
                        wexb_sb[kc][:, D_OUT : D_OUT + 3 * H],
                        start=(kc == 0),
                        stop=(kc == 1),
                    )
                et = et_pool.tile([128, 3 * H], f32, tag="et")
                nc.vector.tensor_copy(et[:], pe_[:])
                et_sb.append(et)
                eu = eu_pool.tile([128, 2 * H], f32, tag="eu")
                nc.scalar.activation(
                    eu[:], et[:, H : 3 * H], AF.Exp, bias=shift_col[:]
                )
                eu_sb.append(eu)
            # t rows: transpose e1 columns -> [8, 1024], exp, DRAM, broadcast
            e1t = psT.tile([8, N], f32, tag="e1t")
            for c in range(NC_CHUNKS):
                nc.tensor.transpose(
                    e1t[:, c * 128 : (c + 1) * 128], et_sb[c][:, 0:H], ident[:]
                )
            tr = tr_pool.tile([8, N], f16, tag="tr")
            nc.scalar.activation(tr[:], e1t[:], AF.Exp, scale=-(1.0 - ALPHA))
            nc.sync.dma_start(t_scratch[:], tr[:])
            for c in range(NC_CHUNKS):
                p1 = psB.tile([128, D_OUT], f32, tag="mm1")
                for kc in range(2):
                    nc.tensor.matmul(
                        p1[:],
                        htb_sb[kc][:, c * 128 : (c + 1) * 128],
                        wexb_sb[kc][:, 0:D_OUT],
                        start=(kc == 0),
                        stop=(kc == 1),
                    )
                wt = wh_pool.tile([128, H, HD + 1], f16, tag="wh")
                nc.vector.memset(wt[:, :, 0:1], 1.0)
                nc.scalar.copy(
                    wt[:, :, 1 : HD + 1], p1[:].rearrange("p (h q) -> p h q", h=H)
                )
                wh_sb.append(wt)

        # t broadcast: zero-stride DRAM reads replicate each head's row
        # across all 128 partitions (one DMA per head so head 0 unblocks fast)
        tb_all = tb_pool.tile([128, H, N], f16, tag="tb", name="tb_all")
        for hh in range(H):
            nc.scalar.dma_start(
                tb_all[:, hh, :],
                t_scratch[0:1, hh * N : (hh + 1) * N].partition_broadcast(128),
            )

        # Keep the PE busy across the pre-main-loop lull: a >3.4us idle
        # window re-throttles the PE clock to 1.2GHz (HAM).
        with tc.tile_pool(name="psum_warm", bufs=1, space="PSUM") as psW:
            warm_ps = psW.tile([128, 512], f32, tag="warm_ps")
            for _ in range(12):
                nc.tensor.matmul(
                    warm_ps[:],
                    htb_sb[0][:, 0:128],
                    htb_sb[0][:, 0:512],
                    start=True,
                    stop=True,
                )

        # ---- main loop: scores -> mask -> attention matmul ----
        with tc.tile_pool(name="psum_mm2", bufs=4, space="PSUM") as ps2:
            warm2 = ps2.tile([128, 256], f32, tag="warm2", bufs=1)
            for hh in range(H):
                acc = [
                    ps2.tile([HD + 1, 512], f32, tag="mm2", name=f"acc{hh}_{i}")
                    for i in range(2)
                ]
                for c in range(NC_CHUNKS):
                    q = q_pool.tile([128, N], f16, tag="q")
                    nc.vector.tensor_scalar(
                        q[:],
                        tb_all[:, hh, :],
                        eu_sb[c][:, H + hh : H + hh + 1],
                        eu_sb[c][:, hh : hh + 1],
                        Alu.mult,
                        Alu.max,
                    )
                    g = g_pool.tile([128, N], f16, tag="g")
                    if _mask_on_gpsimd(hh, c):
                        nc.gpsimd.tensor_tensor(
                            out=g[:], in0=q[:], in1=af_sb[c], op=Alu.min
                        )
                    else:
                        nc.vector.tensor_tensor(
                            out=g[:], in0=q[:], in1=af_sb[c], op=Alu.min
                        )
                    for ic in range(2):
                        nc.tensor.matmul(
                            acc[ic][:],
                            wh_sb[c][:, hh, :],
                            g[:, ic * 512 : (ic + 1) * 512],
                            start=(c == 0),
                            stop=(c == NC_CHUNKS - 1),
                        )
                # tiny filler keeps the PE's activity monitor from
                # re-throttling the clock during sub-window idle gaps
                nc.tensor.matmul(
                    warm2[:],
                    htb_sb[0][:, 0:128],
                    htb_sb[0][:, 0:256],
                    start=True,
                    stop=True,
                )
                # evacuate PSUM -> SBUF (fp16) -> DRAM; row 0 is the
                # denominator, rows 1..32 the numerator. Host divides.
                st = st_pool.tile([HD + 1, N], f16, tag="st", name=f"st{hh}")
                nc.scalar.copy(st[:, 0:512], acc[0][:])
                nc.scalar.copy(st[:, 512:1024], acc[1][:])
                nc.sync.dma_start(
                    outd_d[hh * (HD + 1) : (hh + 1) * (HD + 1), :], st[:]
                )

    if split_waits:
        _split_multi_waits(nc)
    return nc


def _get_nc():
    if "nc" not in _NC_CACHE:
        _NC_CACHE["nc"] = _build_nc()
    return _NC_CACHE["nc"]


def _prep_inputs(h, adj_mask, W, a):
    hT = np.ascontiguousarray(np.swapaxes(np.asarray(h, dtype=np.float32), 1, 2))
    adj = np.asarray(adj_mask)
    W = np.asarray(W, dtype=np.float32)
    a = np.asarray(a, dtype=np.float32)

    # multiplicative mask, transposed: af[b, j, i] = 1000 if adj[b, i, j] else 0
    # (1000 > max possible Q, so min(Q, af) = adj * Q exactly)
    af = np.where(
        np.swapaxes(adj, 1, 2) == 0, np.float16(0.0), np.float16(1000.0)
    ).astype(np.float16)

    Wr = W.reshape(D_IN, H, HD)
    w1 = Wr @ a[:HD]  # [D_IN, H] -> e1
    w2 = Wr @ a[HD:]  # [D_IN, H] -> e2
    wex = np.ascontiguousarray(
        np.concatenate([W, w1, w2, ALPHA * w2], axis=1), dtype=np.float32
    )
    import ml_dtypes

    htb = hT.astype(ml_dtypes.bfloat16)
    wexb = wex.astype(ml_dtypes.bfloat16)
    return af, htb, wexb


def kernel(h, adj_mask, W, a):
    global LAST_RESULT
    # persistent jax/XLA cache: repeat calls (and reruns) skip the multi-
    # minute neuronx-cc compile for an unchanged module
    os.environ.setdefault("JAX_COMPILATION_CACHE_DIR", "/tmp/jax_bass_cache")
    from concourse.bass_utils import run_bass_kernel_spmd

    af_np, htb_np, wexb_np = _prep_inputs(h, adj_mask, W, a)
    nc = _get_nc()

    core_ids = list(range(N_CORES))
    in_maps = [
        {
            "htb": np.ascontiguousarray(htb_np[b]),
            "af": np.ascontiguousarray(af_np[b]),
            "wexb": wexb_np,
        }
        for b in range(N_CORES)
    ]
    res = run_bass_kernel_spmd(nc, in_maps, core_ids)
    LAST_RESULT = res
    outs = []
    for b in range(N_CORES):
        o = np.asarray(res.results[b]["outd"]).astype(np.float32)
        o = o.reshape(H, HD + 1, N)
        num = o[:, 1:, :]  # [H, HD, N]
        den = o[:, 0:1, :]  # [H, 1, N]
        outs.append((num / den).transpose(2, 0, 1).reshape(N, D_OUT))
    return np.stack(outs).astype(np.float32)
